# revision 20
# baseline (speedup 1.0000x reference)
"""Trainium2 Bass kernel for nn_EnhancedAttentionLayer (B=4, S=2048, D=1024).

Single-head attention computed in weight-folded form. Because the head is
single and the projections square, the score and value paths contract to

  S  = x (Wq^T Wk) x^T / sqrt(D)          Wqk := Wq^T Wk   (host, once)
  y  = softmax(S) x (Wo Wv)^T             Wvo := (Wo Wv)^T (host, once)

so the per-core device work drops from 22.0 GF (Q/K/V/out projections with
K/V duplicated across the query-split pair) to 13.4 GF: N = xq Wqk (2.15),
S = N x^T (4.29), C = P x (4.29), y = C Wvo (2.15), colsum 0.54. The folded
weight products are x-independent preprocessing done once on host numpy.

Sharding: 8 cores = (batch b in 0..3) x (query-half h in 0..1); every core
sees the full 2048-key batch element (xt/xn) but only its 1024 queries (xq).
No work is duplicated across the pair and no collectives are needed.

Layouts (host pre-transposes; contraction dim always on SBUF partitions):
  xt  = x[b].T        [D, S]   B1 stationary (d on partitions)
  xq  = xt query half [D, SQ]  A1 moving operand
  xn  = x[b]          [S, D]   B2a stationary (k on partitions)
  wqk = Wq.T @ Wk     [D, D]
  wvo = (Wo @ Wv).T   [D, D]
Output yt = y_half.T [D, SQ]; host transposes back and reassembles.

Dataflow per core (all matmuls fp32r, moving dim 512, 227 ns/mm issue):
  A1 : NT[d,q]  = wqk.T @ xq          128 mm, 6-chain PSUM waves; wqk on
                                      the Sync DGE queue, xq+xt on Scalar,
                                      so the PE starts ~2 us in and the
                                      first wave rides the two streams
  B1 : ST[k,q]  = xt.T @ NT ; expT = exp(ST/32) (ACT, fused scale)
       colsum via ones-matrix matmul accumulated in PSUM (replicates the
       per-query sum across partitions -> full-width reciprocal), emitted
       one chain late so it never waits on ACT    256+32 mm, no DMA at all
  B2a: CT[d,q]  = xn.T @ expT ; normalize by bcast (DVE MULT)    256 mm
       xn streamed as 8 per-dc chunks (bufs=4, first 4 prefetched during
       B1), wvo row-chunks interleaved on the same queue
  B2b: ytT[o,q] = wvo.T @ CT -> SBUF copy -> DRAM store          128 mm
Accumulation chains rotate through a 6-bank PSUM pool; B1 colsums use the
other 2 banks. softmax max-subtraction is skipped: |scores| <= ~8 so exp
stays well inside fp32. Biases are zeros by spec; bo applied on host if
nonzero.
"""
import sys

if '/opt/trn_rl_repo' not in sys.path:
    sys.path.insert(0, '/opt/trn_rl_repo')

from contextlib import ExitStack

import numpy as np

import concourse.bacc as bacc_mod
import concourse.mybir as mybir
import concourse.tile as tile
from concourse.bass_utils import run_bass_kernel_spmd

F32 = mybir.dt.float32
F32R = mybir.dt.float32r
BF16 = mybir.dt.bfloat16
EXP = mybir.ActivationFunctionType.Exp
MULT = mybir.AluOpType.mult

B, S, D = 4, 2048, 1024
SQ = 1024           # queries per core
P = 128
NDC = D // P        # 8 chunks over d (rows of wqk/xt, also out-chunks)
NKC = S // P        # 16 key chunks
NQH = SQ // 512     # 2 query column-halves (moving dim 512)

LAST_RESULT = [None]
_CACHE = {}


def build_nc():
    nc = bacc_mod.Bacc("TRN2", target_bir_lowering=False, debug=False)

    xt = nc.dram_tensor("xt", [D, S], BF16, kind="ExternalInput")
    xq = nc.dram_tensor("xq", [D, SQ], BF16, kind="ExternalInput")
    xn = nc.dram_tensor("xn", [S, D], BF16, kind="ExternalInput")
    wqk = nc.dram_tensor("wqk", [D, D], BF16, kind="ExternalInput")
    wvo = nc.dram_tensor("wvo", [D, D], BF16, kind="ExternalInput")
    yt = nc.dram_tensor("yt", [D, SQ], BF16, kind="ExternalOutput")

    def part3(ap):  # [R, C] dram -> [128, R/128, C] (rows on partitions)
        return ap.rearrange("(o i) c -> i o c", i=P)

    with tile.TileContext(nc) as tc, ExitStack() as ctx:
        pers = ctx.enter_context(tc.tile_pool(name="pers", bufs=1))
        ones_f = pers.tile([P, P], F32)
        nc.vector.memset(ones_f[:], 1.0)
        ones128 = pers.tile([P, P], F32R)
        nc.vector.tensor_copy(ones128[:], ones_f[:])
        bcast_sb = pers.tile([P, SQ], F32)
        acc_sb = pers.tile([P, NQH, 512], F32R)  # per-qh colsum partials

        with tc.tile_pool(name="xtp", bufs=1) as xtp, \
             tc.tile_pool(name="ntp", bufs=1) as ntp:
            xt_sb = xtp.tile([P, NDC, S], BF16)    # 64 KB/part
            nt_sb = ntp.tile([P, NDC, SQ], BF16)   # 32 KB/part

            with tc.tile_pool(name="a1", bufs=1) as a1:
                wqk_sb = a1.tile([P, NDC, D], BF16)
                xq_sb = a1.tile([P, NDC, SQ], BF16)
                # two DGE queues in parallel; wqk+xq pace the A1 waves, the
                # xt halves queued behind them land well before B1 needs them
                for c in range(NDC):
                    nc.sync.dma_start(wqk_sb[:, c, :],
                                      wqk[c * P:(c + 1) * P, :])
                    nc.scalar.dma_start(xq_sb[:, c, :],
                                        xq[c * P:(c + 1) * P, :])
                for c in range(NDC):
                    eng = nc.sync if c % 2 == 0 else nc.scalar
                    eng.dma_start(xt_sb[:, c, :], xt[c * P:(c + 1) * P, :])

                # ---- A1: NT[d,q] = wqk.T @ xq ----
                # 16 chains in two waves of 8 (all PSUM banks) so the PE has
                # enough independent work to ride the two input streams.
                # qh-major: wave 1 produces every qh=0 NT chunk, so all the
                # copies B1's first chains consume finish during wave 2's
                # compute and B1 starts the moment A1's matmuls end.
                chains = [(dc, qh) for qh in range(NQH) for dc in range(NDC)]
                with tc.tile_pool(name="a1w", bufs=8, space="PSUM") as a1w:
                    for w0 in range(0, len(chains), 8):
                        wave = chains[w0:w0 + 8]
                        ps = [a1w.tile([P, 512], F32, tag="ps",
                                       name=f"a1ps{w0}_{i}")
                              for i in range(len(wave))]
                        for cc in range(NDC):
                            for i, (dc, qh) in enumerate(wave):
                                nc.tensor.matmul(
                                    ps[i][:],
                                    wqk_sb[:, cc, dc * P:(dc + 1) * P],
                                    xq_sb[:, cc, qh * 512:(qh + 1) * 512],
                                    start=(cc == 0), stop=(cc == NDC - 1))
                        for i, (dc, qh) in enumerate(wave):
                            nc.vector.tensor_copy(
                                nt_sb[:, dc, qh * 512:(qh + 1) * 512],
                                ps[i][:])

            # a1 closed; the to-end-of-kernel pools live on the right side.
            # 6-bank PSUM rotation for every B-phase accumulation chain;
            # +2 banks for the two q-halves' colsum reductions during B1
            mps = ctx.enter_context(
                tc.tile_pool(name="mps", bufs=6, space="PSUM"))
            xnp = ctx.enter_context(
                tc.tile_pool(name="xnp", bufs=4, side="right"))
            # xn rides the GpSimd SWDGE queue: a separate queue+semaphore
            # channel, so these loads never fold into the completion
            # targets B1's xt-dependent weight loads wait on
            xn_tiles = {}
            for dc in range(4):  # prefetched during B1
                xn_tiles[dc] = xnp.tile([P, NKC, P], BF16, tag="xn",
                                        name=f"xn{dc}")
                nc.gpsimd.dma_start(xn_tiles[dc][:],
                                    part3(xn[:, dc * P:(dc + 1) * P]))

            epool = ctx.enter_context(
                tc.tile_pool(name="expt", bufs=1, side="right"))
            expt_sb = epool.tile([P, NKC, SQ], BF16)  # 64 KB/part

            # ---- B1: ST[k,q] = xt.T @ NT -> expT; colsum on DVE ----
            # expT chunks are summed elementwise on DVE into a per-qh f32
            # accumulator (off the PE critical path); a single ones-matmul
            # per q-half then reduces over partitions WITH the broadcast
            # replication, so the PE spends 2 mms on softmax sums, not 32.
            with tc.tile_pool(name="sump", bufs=2, space="PSUM") as sump:
                for qh in range(NQH):
                    q0 = qh * 512
                    for kc in range(NKC):
                        ps_s = mps.tile([P, 512], F32, tag="ps",
                                        name=f"pss{qh}_{kc}")
                        for cc in range(NDC):
                            nc.tensor.matmul(
                                ps_s[:], xt_sb[:, cc, kc * P:(kc + 1) * P],
                                nt_sb[:, cc, q0:q0 + 512],
                                start=(cc == 0), stop=(cc == NDC - 1))
                        nc.scalar.activation(
                            expt_sb[:, kc, q0:q0 + 512], ps_s[:], EXP,
                            scale=1.0 / 32.0)
                        if kc == 0:
                            nc.vector.tensor_copy(
                                acc_sb[:, qh, :], expt_sb[:, 0, q0:q0 + 512])
                        else:
                            nc.vector.tensor_tensor(
                                acc_sb[:, qh, :], acc_sb[:, qh, :],
                                expt_sb[:, kc, q0:q0 + 512],
                                mybir.AluOpType.add)
                    ps_sum = sump.tile([P, 512], F32, tag="pssum")
                    nc.tensor.matmul(ps_sum[:], ones128[:], acc_sb[:, qh, :],
                                     start=True, stop=True)
                    # sums replicated on every partition -> full-width recip
                    nc.vector.reciprocal(bcast_sb[:, q0:q0 + 512], ps_sum[:])

        # xt/nt pools closed; their space hosts ct and wvo
        with tc.tile_pool(name="b2", bufs=1) as b2:
            ct_sb = b2.tile([P, NDC, SQ], BF16)    # 32 KB/part
            wvo_sb = b2.tile([P, NDC, D], BF16)    # 32 KB/part

            # ---- B2a: CT[d,q] = xn.T @ expT, normalized ----
            for dc in range(NDC):
                if dc not in xn_tiles:
                    xn_tiles[dc] = xnp.tile([P, NKC, P], BF16, tag="xn",
                                            name=f"xn{dc}")
                    nc.gpsimd.dma_start(xn_tiles[dc][:],
                                        part3(xn[:, dc * P:(dc + 1) * P]))
                xn_t = xn_tiles[dc]
                # wvo row-chunk rides the same queue between xn chunks
                nc.sync.dma_start(wvo_sb[:, dc, :],
                                  wvo[dc * P:(dc + 1) * P, :])
                for qh in range(NQH):
                    q0 = qh * 512
                    ps_c = mps.tile([P, 512], F32, tag="ps",
                                    name=f"pc{dc}_{qh}")
                    for kc in range(NKC):
                        nc.tensor.matmul(
                            ps_c[:], xn_t[:, kc, :],
                            expt_sb[:, kc, q0:q0 + 512],
                            start=(kc == 0), stop=(kc == NKC - 1))
                    nc.vector.tensor_tensor(
                        ct_sb[:, dc, q0:q0 + 512], ps_c[:],
                        bcast_sb[:, q0:q0 + 512], MULT)

            # ---- B2b: ytT[o,q] = wvo.T @ CT ----
            with tc.tile_pool(name="b2y", bufs=3) as b2y:
                for oc in range(NDC):
                    for qh in range(NQH):
                        q0 = qh * 512
                        ps_o = mps.tile([P, 512], F32, tag="ps",
                                        name=f"po{oc}_{qh}")
                        for cc in range(NDC):
                            nc.tensor.matmul(
                                ps_o[:], wvo_sb[:, cc, oc * P:(oc + 1) * P],
                                ct_sb[:, cc, q0:q0 + 512],
                                start=(cc == 0), stop=(cc == NDC - 1))
                        yst = b2y.tile([P, 512], BF16, tag="yst")
                        nc.vector.tensor_copy(yst[:], ps_o[:])
                        nc.scalar.dma_start(
                            yt[oc * P:(oc + 1) * P, q0:q0 + 512], yst[:])

    nc.compile()
    return nc


def _get_nc():
    if "nc" not in _CACHE:
        _CACHE["nc"] = build_nc()
    return _CACHE["nc"]


def kernel(x, Wq, bq, Wk, bk, Wv, bv, Wo, bo, _trace=False):
    import ml_dtypes
    bf16 = ml_dtypes.bfloat16
    x = np.asarray(x, dtype=np.float32)
    Wq = np.asarray(Wq, dtype=np.float32)
    Wk = np.asarray(Wk, dtype=np.float32)
    Wv = np.asarray(Wv, dtype=np.float32)
    Wo = np.asarray(Wo, dtype=np.float32)
    wqk = np.ascontiguousarray((Wq.T @ Wk).astype(bf16))
    wvo = np.ascontiguousarray(((Wo @ Wv).T).astype(bf16))
    xb = np.ascontiguousarray(x.astype(bf16))

    in_maps = []
    xts = {}
    for c in range(8):
        b, h = c // 2, c % 2
        if b not in xts:
            xts[b] = np.ascontiguousarray(xb[b].T)
        xt = xts[b]
        xq = np.ascontiguousarray(xt[:, h * SQ:(h + 1) * SQ])
        in_maps.append({"xt": xt, "xq": xq, "xn": xb[b], "wqk": wqk,
                        "wvo": wvo})

    nc = _get_nc()
    kw = {}
    if _trace:
        kw = dict(trace=True, stitch_traces=False)
    res = run_bass_kernel_spmd(nc, in_maps, core_ids=list(range(8)), **kw)
    LAST_RESULT[0] = res

    y = np.empty((B, S, D), dtype=np.float32)
    for c in range(8):
        b, h = c // 2, c % 2
        y[b, h * SQ:(h + 1) * SQ, :] = res.results[c]["yt"].T.astype(
            np.float32)

    bo = np.asarray(bo, dtype=np.float32)
    if bo.any():
        y = y + bo
    return y


# revision 23
# speedup vs baseline: 1.0443x; 1.0443x over previous
"""Trainium2 Bass kernel for nn_EnhancedAttentionLayer (B=4, S=2048, D=1024).

Single-head attention computed in weight-folded form. Because the head is
single and the projections square, the score and value paths contract to

  S  = x (Wq^T Wk) x^T / sqrt(D)          Wqk := Wq^T Wk   (host, once)
  y  = softmax(S) x (Wo Wv)^T             Wvo := (Wo Wv)^T (host, once)

so the per-core device work drops from 22.0 GF (Q/K/V/out projections with
K/V duplicated across the query-split pair) to 13.4 GF: N = xq Wqk (2.15),
S = N x^T (4.29), C = P x (4.29), y = C Wvo (2.15), colsum 0.54. The folded
weight products are x-independent preprocessing done once on host numpy.

Sharding: 8 cores = (batch b in 0..3) x (query-half h in 0..1); every core
sees the full 2048-key batch element (xt/xn) but only its 1024 queries (xq).
No work is duplicated across the pair and no collectives are needed.

Layouts (host pre-transposes; contraction dim always on SBUF partitions):
  xt  = x[b].T        [D, S]   B1 stationary (d on partitions)
  xq  = xt query half [D, SQ]  A1 moving operand
  xn  = x[b]          [S, D]   B2a stationary (k on partitions)
  wqk = Wq.T @ Wk     [D, D]
  wvo = (Wo @ Wv).T   [D, D]
Output yt = y_half.T [D, SQ]; host transposes back and reassembles.

Dataflow per core (all matmuls fp32r, moving dim 512, 227 ns/mm issue):
  A1 : NT[d,q]  = wqk.T @ xq          128 mm, 6-chain PSUM waves; wqk on
                                      the Sync DGE queue, xq+xt on Scalar,
                                      so the PE starts ~2 us in and the
                                      first wave rides the two streams
  B1 : ST[k,q]  = xt.T @ NT ; expT = exp(ST/32) (ACT, fused scale)
       colsum via ones-matrix matmul accumulated in PSUM (replicates the
       per-query sum across partitions -> full-width reciprocal), emitted
       one chain late so it never waits on ACT    256+32 mm, no DMA at all
  B2a: CT[d,q]  = xn.T @ expT ; normalize by bcast (DVE MULT)    256 mm
       xn streamed as 8 per-dc chunks (bufs=4, first 4 prefetched during
       B1), wvo row-chunks interleaved on the same queue
  B2b: ytT[o,q] = wvo.T @ CT -> SBUF copy -> DRAM store          128 mm
Accumulation chains rotate through a 6-bank PSUM pool; B1 colsums use the
other 2 banks. softmax max-subtraction is skipped: |scores| <= ~8 so exp
stays well inside fp32. Biases are zeros by spec; bo applied on host if
nonzero.
"""
import sys

if '/opt/trn_rl_repo' not in sys.path:
    sys.path.insert(0, '/opt/trn_rl_repo')

from contextlib import ExitStack

import numpy as np

import concourse.bacc as bacc_mod
import concourse.mybir as mybir
import concourse.tile as tile
from concourse.bass_utils import run_bass_kernel_spmd

F32 = mybir.dt.float32
F32R = mybir.dt.float32r
BF16 = mybir.dt.bfloat16
EXP = mybir.ActivationFunctionType.Exp
MULT = mybir.AluOpType.mult

B, S, D = 4, 2048, 1024
SQ = 1024           # queries per core
P = 128
NDC = D // P        # 8 chunks over d (rows of wqk/xt, also out-chunks)
NKC = S // P        # 16 key chunks
NQH = SQ // 512     # 2 query column-halves (moving dim 512)

LAST_RESULT = [None]
_CACHE = {}


def build_nc():
    nc = bacc_mod.Bacc("TRN2", target_bir_lowering=False, debug=False)

    xt = nc.dram_tensor("xt", [D, S], BF16, kind="ExternalInput")
    xq = nc.dram_tensor("xq", [D, SQ], BF16, kind="ExternalInput")
    xn = nc.dram_tensor("xn", [S, D], BF16, kind="ExternalInput")
    wqk = nc.dram_tensor("wqk", [D, D], BF16, kind="ExternalInput")
    wvo = nc.dram_tensor("wvo", [D, D], BF16, kind="ExternalInput")
    yt = nc.dram_tensor("yt", [D, SQ], BF16, kind="ExternalOutput")

    def part3(ap):  # [R, C] dram -> [128, R/128, C] (rows on partitions)
        return ap.rearrange("(o i) c -> i o c", i=P)

    with tile.TileContext(nc) as tc, ExitStack() as ctx:
        pers = ctx.enter_context(tc.tile_pool(name="pers", bufs=1))
        ones_f = pers.tile([P, P], F32)
        nc.vector.memset(ones_f[:], 1.0)
        ones128 = pers.tile([P, P], F32R)
        nc.vector.tensor_copy(ones128[:], ones_f[:])
        bcast_sb = pers.tile([P, SQ], F32)
        acc_sb = pers.tile([P, NQH, 512], F32R)  # per-qh colsum partials

        with tc.tile_pool(name="xtp", bufs=1) as xtp, \
             tc.tile_pool(name="ntp", bufs=1) as ntp:
            xt_sb = xtp.tile([P, NDC, S], BF16)    # 64 KB/part
            nt_sb = ntp.tile([P, NDC, SQ], BF16)   # 32 KB/part

            with tc.tile_pool(name="a1", bufs=1) as a1:
                wqk_sb = a1.tile([P, NDC, D], BF16)
                xq_sb = a1.tile([P, NDC, SQ], BF16)
                # two DGE queues in parallel; wqk+xq pace the A1 waves, the
                # xt halves queued behind them land well before B1 needs them
                for c in range(NDC):
                    nc.sync.dma_start(wqk_sb[:, c, :],
                                      wqk[c * P:(c + 1) * P, :])
                    nc.scalar.dma_start(xq_sb[:, c, :],
                                        xq[c * P:(c + 1) * P, :])
                for c in range(NDC):
                    eng = nc.sync if c % 2 == 0 else nc.scalar
                    eng.dma_start(xt_sb[:, c, :], xt[c * P:(c + 1) * P, :])

                # ---- A1: NT[d,q] = wqk.T @ xq ----
                # 16 chains in two waves of 8 (all PSUM banks) so the PE has
                # enough independent work to ride the two input streams.
                # qh-major: wave 1 produces every qh=0 NT chunk, so all the
                # copies B1's first chains consume finish during wave 2's
                # compute and B1 starts the moment A1's matmuls end.
                chains = [(dc, qh) for qh in range(NQH) for dc in range(NDC)]
                with tc.tile_pool(name="a1w", bufs=8, space="PSUM") as a1w:
                    for w0 in range(0, len(chains), 8):
                        wave = chains[w0:w0 + 8]
                        ps = [a1w.tile([P, 512], F32, tag="ps",
                                       name=f"a1ps{w0}_{i}")
                              for i in range(len(wave))]
                        for cc in range(NDC):
                            for i, (dc, qh) in enumerate(wave):
                                nc.tensor.matmul(
                                    ps[i][:],
                                    wqk_sb[:, cc, dc * P:(dc + 1) * P],
                                    xq_sb[:, cc, qh * 512:(qh + 1) * 512],
                                    start=(cc == 0), stop=(cc == NDC - 1))
                        for i, (dc, qh) in enumerate(wave):
                            nc.vector.tensor_copy(
                                nt_sb[:, dc, qh * 512:(qh + 1) * 512],
                                ps[i][:])

            # a1 closed; the to-end-of-kernel pools live on the right side.
            # 6-bank PSUM rotation for every B-phase accumulation chain;
            # +2 banks for the two q-halves' colsum reductions during B1
            mps = ctx.enter_context(
                tc.tile_pool(name="mps", bufs=6, space="PSUM"))
            xnp = ctx.enter_context(
                tc.tile_pool(name="xnp", bufs=4, side="right"))
            xn_tiles = {}

            epool = ctx.enter_context(
                tc.tile_pool(name="expt", bufs=1, side="right"))
            expt_sb = epool.tile([P, NKC, SQ], BF16)  # 64 KB/part

            # ---- B1: ST[k,q] = xt.T @ NT -> expT; colsum on DVE ----
            # expT chunks are summed elementwise on DVE into a per-qh f32
            # accumulator (off the PE critical path); a single ones-matmul
            # per q-half then reduces over partitions WITH the broadcast
            # replication, so the PE spends 2 mms on softmax sums, not 32.
            with tc.tile_pool(name="sump", bufs=2, space="PSUM") as sump:
                for qh in range(NQH):
                    q0 = qh * 512
                    for kc in range(NKC):
                        ps_s = mps.tile([P, 512], F32, tag="ps",
                                        name=f"pss{qh}_{kc}")
                        for cc in range(NDC):
                            nc.tensor.matmul(
                                ps_s[:], xt_sb[:, cc, kc * P:(kc + 1) * P],
                                nt_sb[:, cc, q0:q0 + 512],
                                start=(cc == 0), stop=(cc == NDC - 1))
                        nc.scalar.activation(
                            expt_sb[:, kc, q0:q0 + 512], ps_s[:], EXP,
                            scale=1.0 / 32.0)
                        if kc == 0:
                            nc.vector.tensor_copy(
                                acc_sb[:, qh, :], expt_sb[:, 0, q0:q0 + 512])
                        else:
                            nc.vector.tensor_tensor(
                                acc_sb[:, qh, :], acc_sb[:, qh, :],
                                expt_sb[:, kc, q0:q0 + 512],
                                mybir.AluOpType.add)
                    ps_sum = sump.tile([P, 512], F32, tag="pssum")
                    nc.tensor.matmul(ps_sum[:], ones128[:], acc_sb[:, qh, :],
                                     start=True, stop=True)
                    # sums replicated on every partition -> full-width recip
                    nc.vector.reciprocal(bcast_sb[:, q0:q0 + 512], ps_sum[:])

            # xn prefetch, issued AFTER B1's matmuls on purpose: B1's
            # xt-gated weight loads wait on DMA-completion targets that
            # fold in everything issued earlier on the queue, so putting
            # these first would stall B1 until they land. The Sync engine
            # still reaches them right after the xt chunks, so all four
            # stream in during B1 with zero contention against A1.
            for dc in range(4):
                xn_tiles[dc] = xnp.tile([P, NKC, P], BF16, tag="xn",
                                        name=f"xn{dc}")
                nc.sync.dma_start(xn_tiles[dc][:],
                                  part3(xn[:, dc * P:(dc + 1) * P]))

        # xt/nt pools closed; their space hosts ct and wvo
        with tc.tile_pool(name="b2", bufs=1) as b2:
            ct_sb = b2.tile([P, NDC, SQ], BF16)    # 32 KB/part
            wvo_sb = b2.tile([P, NDC, D], BF16)    # 32 KB/part

            # ---- B2a: CT[d,q] = xn.T @ expT, normalized ----
            for dc in range(NDC):
                if dc not in xn_tiles:
                    xn_tiles[dc] = xnp.tile([P, NKC, P], BF16, tag="xn",
                                            name=f"xn{dc}")
                    nc.sync.dma_start(xn_tiles[dc][:],
                                      part3(xn[:, dc * P:(dc + 1) * P]))
                xn_t = xn_tiles[dc]
                # wvo row-chunk rides the same queue between xn chunks
                nc.sync.dma_start(wvo_sb[:, dc, :],
                                  wvo[dc * P:(dc + 1) * P, :])
                for qh in range(NQH):
                    q0 = qh * 512
                    ps_c = mps.tile([P, 512], F32, tag="ps",
                                    name=f"pc{dc}_{qh}")
                    for kc in range(NKC):
                        nc.tensor.matmul(
                            ps_c[:], xn_t[:, kc, :],
                            expt_sb[:, kc, q0:q0 + 512],
                            start=(kc == 0), stop=(kc == NKC - 1))
                    nc.vector.tensor_tensor(
                        ct_sb[:, dc, q0:q0 + 512], ps_c[:],
                        bcast_sb[:, q0:q0 + 512], MULT)

            # ---- B2b: ytT[o,q] = wvo.T @ CT ----
            with tc.tile_pool(name="b2y", bufs=3) as b2y:
                for oc in range(NDC):
                    for qh in range(NQH):
                        q0 = qh * 512
                        ps_o = mps.tile([P, 512], F32, tag="ps",
                                        name=f"po{oc}_{qh}")
                        for cc in range(NDC):
                            nc.tensor.matmul(
                                ps_o[:], wvo_sb[:, cc, oc * P:(oc + 1) * P],
                                ct_sb[:, cc, q0:q0 + 512],
                                start=(cc == 0), stop=(cc == NDC - 1))
                        yst = b2y.tile([P, 512], BF16, tag="yst")
                        nc.vector.tensor_copy(yst[:], ps_o[:])
                        nc.scalar.dma_start(
                            yt[oc * P:(oc + 1) * P, q0:q0 + 512], yst[:])

    nc.compile()
    return nc


def _get_nc():
    if "nc" not in _CACHE:
        _CACHE["nc"] = build_nc()
    return _CACHE["nc"]


def kernel(x, Wq, bq, Wk, bk, Wv, bv, Wo, bo, _trace=False):
    import ml_dtypes
    bf16 = ml_dtypes.bfloat16
    x = np.asarray(x, dtype=np.float32)
    Wq = np.asarray(Wq, dtype=np.float32)
    Wk = np.asarray(Wk, dtype=np.float32)
    Wv = np.asarray(Wv, dtype=np.float32)
    Wo = np.asarray(Wo, dtype=np.float32)
    wqk = np.ascontiguousarray((Wq.T @ Wk).astype(bf16))
    wvo = np.ascontiguousarray(((Wo @ Wv).T).astype(bf16))
    xb = np.ascontiguousarray(x.astype(bf16))

    in_maps = []
    xts = {}
    for c in range(8):
        b, h = c // 2, c % 2
        if b not in xts:
            xts[b] = np.ascontiguousarray(xb[b].T)
        xt = xts[b]
        xq = np.ascontiguousarray(xt[:, h * SQ:(h + 1) * SQ])
        in_maps.append({"xt": xt, "xq": xq, "xn": xb[b], "wqk": wqk,
                        "wvo": wvo})

    nc = _get_nc()
    kw = {}
    if _trace:
        kw = dict(trace=True, stitch_traces=False)
    res = run_bass_kernel_spmd(nc, in_maps, core_ids=list(range(8)), **kw)
    LAST_RESULT[0] = res

    y = np.empty((B, S, D), dtype=np.float32)
    for c in range(8):
        b, h = c // 2, c % 2
        y[b, h * SQ:(h + 1) * SQ, :] = res.results[c]["yt"].T.astype(
            np.float32)

    bo = np.asarray(bo, dtype=np.float32)
    if bo.any():
        y = y + bo
    return y
